# revision 11
# baseline (speedup 1.0000x reference)
"""Trainium2 Bass kernel for nn_Attention_32839319945876 (sparse_attention).

Head-parallel: 48 heads -> 6 per core on 8 NeuronCores. The axon tunnel
charges per argument buffer and per argument byte on every call, so the
design minimizes wire bytes:

- One flat bf16-typed wire blob per core; segments bitcast to
  f32/fp16/fp8 at consumption.
- Selection-path tensors (A, X, WQ, WK) ship as fp16 hi + e4m3 residual
  (3 B/elem, ~2^-18 effective rounding; measured 3.5e-3 end-to-end).
- A/X and all shared constants ship as 1/8 slices, AllGathered on
  device across all 8 cores. (WK's parity half ships per-core: a
  grouped-replica {0,2,4,6}/{1,3,5,7} AllGather desyncs the mesh on
  repeated execution, measured.)
- The wedge matrices (Mqdup/Mkdup) are built on device from the tiny
  generators (64x64 skew + per-head bias) instead of shipping 6 dense
  128x128 mats per core.
- The folded K projection is computed on device as kf = kp @ M per
  head (k_wedged = k_vanilla @ M), removing the second K weight matrix
  entirely.
- Output returns fp16 (1.2e-4 rel).

Precision: the old kernel's 1.7e-2 error came from f32r OPERAND
STORAGE, which rounds to ~2^-13 (measured) and flips near-tie top-12
selections. The whole score path here uses plain F32 tiles/matmuls
(exact to ~1e-7, measured; the extra PE passes are free at this
scale). Top-12 is thresholded on raw f32 scores via max8 +
match_replace + max8 -> 12th largest; exp is only applied to the
selected weights. Values-path internals run fp16 instead of bf16
(same speed, 8x mantissa).
"""

import math
import sys
import types

import numpy as np
import ml_dtypes

try:
    import antenv.axon_hooks  # noqa: F401
except Exception:
    _m = types.ModuleType("antenv.axon_hooks")
    _m.get_axon_ntff_profile_hook = lambda: None
    sys.modules["antenv.axon_hooks"] = _m

import contextlib

import concourse.bass as bass
import concourse.bacc as bacc
import concourse.tile as tile
from concourse import mybir
from concourse.bass_utils import run_bass_kernel_spmd

B, T, C = 1, 1024, 768
N_HEAD = 12
N_BR = 4
DH = C // N_HEAD          # 64
H_TOT = N_BR * N_HEAD     # 48
K_RET = 12
EPS = 1.1920929e-07
N_CORES = 8
HPC = H_TOT // N_CORES    # 6
NTB = T // 128            # 8
NCH = C // 128            # 6
NPAIR = HPC // 2          # 3
SCALE = DH ** -0.5
SIG_C = math.pi / math.sqrt(3.0)
NEG = -40.0
SAX = 16                  # residual scale exponent for A/X
SW = 21                   # residual scale exponent for WQ/WK

F32 = mybir.dt.float32
F32R = mybir.dt.float32r
BF16 = mybir.dt.bfloat16
FP16 = mybir.dt.float16
FP8 = mybir.dt.float8e4
ACTF = mybir.ActivationFunctionType
ALU = mybir.AluOpType

_DUPCOL = np.concatenate([np.arange(0, DH, 2), np.arange(1, DH, 2),
                          np.arange(0, DH, 2), np.arange(1, DH, 2)])

# ---------------------------------------------------------------------------
# Wire layout (all offsets/sizes in bf16 units; every segment even-sized so
# f32 bitcasts stay 4-byte aligned).
# G8: 8-way AllGather section. Entries: (name, rows, width, bytes/elem).
_G8_LAYOUT = [
    ("A_hi", C, T, 2), ("X_hi", C, T, 2),
    ("A_lo", C, T, 1), ("X_lo", C, T, 1),
    ("FR", 128, T, 4), ("TRI", 128, 128, 4), ("PADD", 128, DH, 4),
    ("DUP", DH, 128, 4), ("GQ", DH, DH, 4), ("GK", DH, DH, 4),
    ("FCW", DH, 4 * DH, 2), ("PJW", 4 * DH, DH, 2),
]
# No grouped-replica collectives: a 4-way AllGather desyncs the mesh on
# repeated execution (measured), so WK's parity half ships per-core.
_G4_LAYOUT = []
# PC: per-core section. (name, rows, width, bytes/elem)
_PC_LAYOUT = [
    ("IDE16", 128, 128, 2),
    ("WK_hi", C, HPC * DH, 2), ("WK_lo", C, HPC * DH, 1),
    ("WQ_hi", C, HPC * DH, 2), ("WQ_lo", C, HPC * DH, 1),
    ("WOP", NPAIR * 128, C, 2),
    ("BIASQ", DH, 8, 4), ("BIASK", DH, 8, 4),
    ("WQB", 1, HPC * DH, 4), ("WKB", 1, HPC * DH, 4), ("ESINK", 1, 8, 4),
    ("VSINK", 1, HPC * DH, 2), ("FCB", 1, 4 * DH, 2), ("PJB", 1, DH, 2),
    ("YB", 1, C, 2),
]


def _lay(layout, nshard):
    segs, off = {}, 0
    for name, rows, width, bpe in layout:
        assert rows % nshard == 0
        csize = (rows // nshard) * width * bpe // 2
        assert (rows // nshard) * width * bpe % 4 == 0
        segs[name] = (off, rows, width, bpe)
        off += csize
    return segs, off


_G8_SEGS, G8CH = _lay(_G8_LAYOUT, 8)
_G4_SEGS, G4CH = _lay(_G4_LAYOUT, 4)
_PC_SEGS, PCCH = _lay(_PC_LAYOUT, 1)
W0_TOT = G8CH + G4CH + PCCH

_DT_NP = {2: np.float16, 1: ml_dtypes.float8_e4m3, 4: np.float32}
_DT_MY = {2: FP16, 1: FP8, 4: F32}


def _enc3(x, sexp):
    """fp16 hi + e4m3 residual (240-clipped) at scale 2**sexp."""
    x = np.ascontiguousarray(x, dtype=np.float32)
    hi = x.astype(np.float16)
    r = (x - hi.astype(np.float32)) * (2.0 ** sexp)
    lo = np.clip(r, -240.0, 240.0).astype(ml_dtypes.float8_e4m3)
    return hi, lo


def _rope_tables():
    inv = 1.0 / (10000.0 ** (np.arange(0, DH, 2, dtype=np.float64) / DH))
    ang = np.arange(T, dtype=np.float64)[:, None] * inv[None, :]
    cos, sin = np.cos(ang), np.sin(ang)
    F = np.concatenate([cos.T, -sin.T, sin.T, cos.T], axis=0)
    return F.astype(np.float32)


def _pair_add():
    P = np.zeros((128, DH), np.float32)
    for m in range(32):
        P[m, m] = 1.0
        P[m + 32, m] = 1.0
    for m in range(32, 64):
        P[m + 32, m] = 1.0
        P[m + 64, m] = 1.0
    return P


def _pack(parts_by_name, layout, shard_slice):
    """Pack (possibly sharded) segments into one bf16-viewed flat array."""
    out = []
    for name, rows, width, bpe in layout:
        a = parts_by_name[name]
        a = a[shard_slice(rows)] if shard_slice else a
        a = np.ascontiguousarray(a)
        assert a.dtype == _DT_NP[bpe], (name, a.dtype)
        out.append(a.reshape(-1).view(np.uint8))
    return np.concatenate(out).view(ml_dtypes.bfloat16)


def _host_prep(A, X, WK_w, WK_b, WQ_w, WQ_b, wedge_A, wedge_bias, sink,
               v_nulls, fc_w, fc_b, proj_w, proj_b, WO, WO_b):
    A = np.asarray(A, np.float32)[0]
    X = np.asarray(X, np.float32)[0]
    WK_w = np.asarray(WK_w, np.float32)
    WK_b = np.asarray(WK_b, np.float32)
    WQ_w = np.asarray(WQ_w, np.float32)
    WQ_b = np.asarray(WQ_b, np.float32)
    wedge_A = np.asarray(wedge_A, np.float32)
    wedge_bias = np.asarray(wedge_bias, np.float32)
    sink = np.asarray(sink, np.float32)
    v_nulls = np.asarray(v_nulls, np.float32)
    fc_w = np.asarray(fc_w, np.float32)
    fc_b = np.asarray(fc_b, np.float32)
    proj_w = np.asarray(proj_w, np.float32)
    proj_b = np.asarray(proj_b, np.float32)
    WO = np.asarray(WO, np.float32)
    WO_b = np.asarray(WO_b, np.float32)

    AT_hi, AT_lo = _enc3(A.T, SAX)
    XT_hi, XT_lo = _enc3(X.T, SAX)
    Askew = wedge_A - wedge_A.T
    D = np.zeros((DH, 128), np.float32)
    D[_DUPCOL, np.arange(128)] = 1.0
    g8_common = {
        "A_hi": AT_hi, "X_hi": XT_hi, "A_lo": AT_lo, "X_lo": XT_lo,
        "FR": _rope_tables(),
        "TRI": np.where(np.tril(np.ones((128, 128), bool)), 0.0,
                        NEG).astype(np.float32),
        "PADD": _pair_add(),
        "DUP": D,
        "GQ": ((np.eye(DH, dtype=np.float32) - Askew) * SCALE).astype(
            np.float32),
        "GK": (np.eye(DH, dtype=np.float32) - Askew).astype(np.float32),
        "FCW": fc_w.astype(np.float16),
        "PJW": proj_w.astype(np.float16),
    }
    ide = np.eye(128, dtype=np.float16)
    vre = v_nulls.reshape(H_TOT, DH)
    esink_all = np.exp(sink.astype(np.float64)).astype(np.float32)

    in_maps = []
    for core in range(N_CORES):
        heads = list(range(core * HPC, (core + 1) * HPC))
        par = core % 2
        wk_half = WK_w[:, par * (C // 2):(par + 1) * (C // 2)]
        wkh_hi, wkh_lo = _enc3(wk_half, SW)
        # PC
        WQ6 = np.concatenate(
            [WQ_w[:, h * DH:(h + 1) * DH] for h in heads], 1)
        wq_hi, wq_lo = _enc3(WQ6, SW)
        WQb6 = np.concatenate([WQ_b[h * DH:(h + 1) * DH] for h in heads])
        WKb6 = WK_b[par * (C // 2):(par + 1) * (C // 2)]
        biasq = np.zeros((DH, 8), np.float32)
        biask = np.zeros((DH, 8), np.float32)
        for j, h in enumerate(heads):
            biasq[:, j] = wedge_bias[h] * SCALE
            biask[:, j] = wedge_bias[h]
        WOp = []
        for p in range(NPAIR):
            h0, h1 = heads[2 * p], heads[2 * p + 1]
            s0 = WO[h0 // N_HEAD][(h0 % N_HEAD) * DH:(h0 % N_HEAD + 1) * DH]
            s1 = WO[h1 // N_HEAD][(h1 % N_HEAD) * DH:(h1 % N_HEAD + 1) * DH]
            WOp.append(np.concatenate([s0, s1], 0) * 0.25)
        WOp = np.stack(WOp).astype(np.float16)
        esink = np.zeros((1, 8), np.float32)
        esink[0, :HPC] = esink_all[heads]
        vsink = np.concatenate([vre[h] * esink_all[h] for h in heads])
        yb = (WO_b.mean(0) if core == 0 else np.zeros(C)).astype(np.float16)
        pc = {
            "IDE16": ide,
            "WK_hi": wkh_hi, "WK_lo": wkh_lo,
            "WQ_hi": wq_hi, "WQ_lo": wq_lo, "WOP": WOp,
            "BIASQ": biasq, "BIASK": biask,
            "WQB": WQb6.reshape(1, -1).astype(np.float32),
            "WKB": WKb6.reshape(1, -1).astype(np.float32),
            "ESINK": esink,
            "VSINK": vsink.reshape(1, -1).astype(np.float16),
            "FCB": fc_b.reshape(1, -1).astype(np.float16),
            "PJB": proj_b.reshape(1, -1).astype(np.float16),
            "YB": yb.reshape(1, -1),
        }
        w0 = np.concatenate([
            _pack(g8_common, _G8_LAYOUT,
                  lambda rows: slice(core * rows // 8,
                                     (core + 1) * rows // 8)),
            _pack(pc, _PC_LAYOUT, None),
        ])
        assert w0.size == W0_TOT, (w0.size, W0_TOT)
        in_maps.append({"w0": w0})
    return in_maps


def build_kernel():
    nc = bacc.Bacc(target_bir_lowering=False, debug=False)
    w0 = nc.declare_dram_parameter("w0", [W0_TOT], BF16, isOutput=False)
    out = nc.declare_dram_parameter("out", [C // N_CORES, T], FP16,
                                    isOutput=True)
    y_bounce = nc.dram_tensor("y_bounce", [C, T], F32)
    y_rs = nc.dram_tensor("y_rs", [C // N_CORES, T], F32)
    g8_in = nc.dram_tensor("g8_in", [G8CH], BF16)
    g8_full = nc.dram_tensor("g8_full", [8 * G8CH], BF16)

    def pc_ap(name, r0=None, r1=None):
        off, rows, width, bpe = _PC_SEGS[name]
        base = G8CH + G4CH + off
        if r0 is None:
            r0, r1 = 0, rows
        lo = base + r0 * width * bpe // 2
        hi = base + r1 * width * bpe // 2
        return w0[lo:hi].bitcast(_DT_MY[bpe])

    def gath_ranges(segs, chsz, nshard, full, name, r0, r1):
        """Yield (dst_row, nrows, dram_ap) covering global rows [r0, r1)."""
        off, rows, width, bpe = segs[name]
        rc = rows // nshard
        dst = 0
        r = r0
        while r < r1:
            blk, o = divmod(r, rc)
            take = min(rc - o, r1 - r)
            lo = blk * chsz + off + o * width * bpe // 2
            hi = lo + take * width * bpe // 2
            yield dst, take, full[lo:hi].bitcast(_DT_MY[bpe])
            dst += take
            r += take

    def g8_ap(name, r0, r1):
        return gath_ranges(_G8_SEGS, G8CH, 8, g8_full, name, r0, r1)

    def pc_ranges(name, r0, r1):
        off, rows, width, bpe = _PC_SEGS[name]
        base = G8CH + G4CH + off
        lo = base + r0 * width * bpe // 2
        hi = base + r1 * width * bpe // 2
        yield 0, r1 - r0, w0[lo:hi].bitcast(_DT_MY[bpe])

    with tile.TileContext(nc) as tc:
        ctx = contextlib.ExitStack()
        with ctx:
            nc.sync.dma_start(g8_in[:], w0[0:G8CH])
            nc.gpsimd.collective_compute(
                "AllGather", ALU.bypass,
                ins=[g8_in.ap().opt()], outs=[g8_full.ap().opt()],
                replica_groups=[list(range(N_CORES))])

            cpool = ctx.enter_context(tc.tile_pool(name="consts", bufs=1))
            wpool = ctx.enter_context(tc.tile_pool(name="weights", bufs=1))
            persist = ctx.enter_context(tc.tile_pool(name="persist", bufs=1))
            work = ctx.enter_context(tc.tile_pool(name="work", bufs=2))
            ework = ctx.enter_context(tc.tile_pool(name="ework", bufs=2))
            tiny = ctx.enter_context(tc.tile_pool(name="tiny", bufs=4))
            dec = ctx.enter_context(tc.tile_pool(name="dec", bufs=2))
            ps_junk = ctx.enter_context(
                tc.tile_pool(name="ps_junk", bufs=1, space="PSUM"))

            junk_ps = ps_junk.tile([32, 32], BF16)

            # identity tiles (PC section: no gather dependency)
            ident_h = cpool.tile([128, 128], FP16, name="ident_h")
            nc.sync.dma_start(ident_h[:], pc_ap("IDE16"))
            ident_b = cpool.tile([128, 128], BF16, name="ident_b")
            nc.scalar.copy(ident_b[:], ident_h[:])
            ident_f = cpool.tile([128, 128], F32, name="ident_f")
            nc.scalar.copy(ident_f[:], ident_h[:])

            def presync_w(psum_ap):
                nc.tensor.matmul(psum_ap.bitcast(BF16)[0:32, 0:32],
                                 ident_b[0:32, 0:32], ident_b[0:32, 0:32],
                                 is_transpose=True, start=True, stop=True)

            def presync_r(ap):
                nc.tensor.matmul(junk_ps[:],
                                 ap.bitcast(BF16)[0:32, 0:32],
                                 ident_b[0:32, 0:32],
                                 is_transpose=True, start=True, stop=True)

            def load_gath(pool, gen, shape, dt, name, tag=None, sync=True):
                kw = {"tag": tag} if tag else {}
                t = pool.tile(list(shape), dt, name=name, **kw)
                for dst, n, ap in gen:
                    nc.sync.dma_start(t[dst:dst + n, :], ap)
                if sync:
                    presync_r(t[:])
                return t

            def load_dec3(pool, hi_gen, lo_gen, shape, sexp, name, tag=None):
                """3-byte decode: f32 = hi16 + lo8 * 2**-sexp.

                Output tiles are plain F32 (NOT f32r: f32r storage rounds
                to ~2^-13, measured, which flips top-12 selections; plain
                f32 matmuls are exact to ~1e-7 and the extra PE passes
                are free at this scale).
                """
                wide = shape[1] == T
                hi_t = dec.tile(list(shape), FP16, name=f"{name}_hi",
                                tag="dhi" if wide else "dhiw")
                lo_t = dec.tile(list(shape), FP8, name=f"{name}_lo",
                                tag="dlo" if wide else "dlow")
                for dst, n, ap in hi_gen:
                    nc.sync.dma_start(hi_t[dst:dst + n, :], ap)
                for dst, n, ap in lo_gen:
                    nc.sync.dma_start(lo_t[dst:dst + n, :], ap)
                kw = {"tag": tag} if tag else {}
                t = pool.tile(list(shape), F32, name=name, **kw)
                nc.vector.scalar_tensor_tensor(
                    t[:], lo_t[:], 2.0 ** -sexp, hi_t[:],
                    ALU.mult, ALU.add)
                presync_r(t[:])
                return t

            # ---------------- constants --------------------------------
            frope_sb = load_gath(cpool, g8_ap("FR", 0, 128), (128, T), F32,
                                 "frope_sb", sync=False)
            tri_sb = load_gath(cpool, g8_ap("TRI", 0, 128), (128, 128), F32,
                               "tri_sb", sync=False)
            padd_sb = load_gath(wpool, g8_ap("PADD", 0, 128), (128, DH), F32,
                                "padd_sb")
            dup_sb = load_gath(wpool, g8_ap("DUP", 0, DH), (DH, 128), F32,
                               "dup_sb")
            gq_sb = load_gath(wpool, g8_ap("GQ", 0, DH), (DH, DH), F32,
                              "gq_sb")
            gk_sb = load_gath(wpool, g8_ap("GK", 0, DH), (DH, DH), F32,
                              "gk_sb")
            fcw_h = load_gath(wpool, g8_ap("FCW", 0, DH), (DH, 4 * DH), FP16,
                              "fcw_h", sync=False)
            pjw_h = [load_gath(wpool, g8_ap("PJW", u * 128, (u + 1) * 128),
                               (128, DH), FP16, f"pjw_h{u}", sync=False)
                     for u in range(2)]

            def pc_row(pool, name, width, dt, nm):
                t = pool.tile([1, width], dt, name=nm)
                nc.sync.dma_start(t[:], pc_ap(name))
                return t

            wqb_f = pc_row(wpool, "WQB", HPC * DH, F32, "wqb_f")
            wkb_f = pc_row(wpool, "WKB", HPC * DH, F32, "wkb_f")
            esink_row = pc_row(cpool, "ESINK", 8, F32, "esink_row")
            vsink_h = pc_row(wpool, "VSINK", HPC * DH, FP16, "vsink_h")
            fcb_h = pc_row(wpool, "FCB", 4 * DH, FP16, "fcb_h")
            pjb_h = pc_row(wpool, "PJB", DH, FP16, "pjb_h")
            yb_h = pc_row(wpool, "YB", C, FP16, "yb_h")
            biasq_sb = wpool.tile([DH, 8], F32, name="biasq_sb")
            nc.sync.dma_start(biasq_sb[:], pc_ap("BIASQ"))
            biask_sb = wpool.tile([DH, 8], F32, name="biask_sb")
            nc.sync.dma_start(biask_sb[:], pc_ap("BIASK"))
            wop_h = [wpool.tile([128, C], FP16, name=f"wop_h{p}")
                     for p in range(NPAIR)]
            for p in range(NPAIR):
                nc.sync.dma_start(wop_h[p][:],
                                  pc_ap("WOP", p * 128, (p + 1) * 128))

            ones_h = cpool.tile([1, T], FP16, name="ones_h")
            nc.vector.memset(ones_h[:], 1.0)
            ones_f = cpool.tile([1, 128], F32, name="ones_f")
            nc.vector.memset(ones_f[:], 1.0)

            # esink broadcast to [128, 8]
            esink_sb = cpool.tile([128, 8], F32, name="esink_sb")

            # ---------------- stage B: projections + transposes --------
            kp_slab = [persist.tile([128, HPC * DH], FP16, name=f"kp{tb}")
                       for tb in range(NTB)]
            qkpool_cm = tc.tile_pool(name="qkpool", bufs=1)
            qkpool = qkpool_cm.__enter__()
            actpool_cm = tc.tile_pool(name="actpool", bufs=1)
            actpool = actpool_cm.__enter__()
            qT = [qkpool.tile([128, T], F32, name=f"qTs{p}")
                  for p in range(NPAIR)]
            kT = [qkpool.tile([128, T], F32, name=f"kTs{p}")
                  for p in range(NPAIR)]
            wq_sb = [load_dec3(qkpool,
                               pc_ranges("WQ_hi", c * 128, (c + 1) * 128),
                               pc_ranges("WQ_lo", c * 128, (c + 1) * 128),
                               (128, HPC * DH), SW, f"wq{c}", tag=f"wx{c}")
                     for c in range(NCH)]

            with tc.tile_pool(name="ps_b", bufs=2, space="PSUM") as ps_b:
                a_sb = [load_dec3(actpool, g8_ap("A_hi", c * 128,
                                                 (c + 1) * 128),
                                  g8_ap("A_lo", c * 128, (c + 1) * 128),
                                  (128, T), SAX, f"at{c}", tag=f"act{c}")
                        for c in range(NCH)]
                # esink broadcast (ones_f column x esink row)
                es_ps = ps_b.tile([128, 8], F32, tag="es", name="es_ps")
                nc.tensor.matmul(es_ps[:], ones_f[:, :],
                                 esink_row[:],
                                 start=True, stop=True)
                nc.vector.tensor_copy(esink_sb[:], es_ps[:])

                # ---- q pass ----
                for tb in range(NTB):
                    ts_ = slice(tb * 128, (tb + 1) * 128)
                    q_ps = ps_b.tile([128, HPC * DH], F32, tag="proj",
                                     name="q_ps")
                    presync_w(q_ps[:])
                    for c in range(NCH):
                        nc.tensor.matmul(q_ps[:], a_sb[c][:, ts_],
                                         wq_sb[c][:],
                                         start=(c == 0), stop=False)
                    nc.tensor.matmul(q_ps[:], ones_f[:, 0:128],
                                     wqb_f[:],
                                     start=False, stop=True)
                    q2 = work.tile([128, HPC * DH], F32, tag="q2", name="q2")
                    nc.scalar.activation(q2[:], q_ps[:], ACTF.Square)
                    ssq = tiny.tile([128, HPC], F32, tag="ssq", name="ssq")
                    nc.vector.reduce_sum(
                        ssq[:], q2[:].rearrange("p (h d) -> p h d", d=DH),
                        axis=mybir.AxisListType.X)
                    nc.vector.tensor_scalar(ssq[:], ssq[:], 1.0 / DH, EPS,
                                            ALU.mult, ALU.add)
                    nc.scalar.activation(ssq[:], ssq[:], ACTF.Sqrt)
                    rin = tiny.tile([128, HPC], F32, tag="rin", name="rin")
                    nc.vector.reciprocal(rin[:], ssq[:])
                    qs = work.tile([128, HPC * DH], F32, tag="qs", name="qs")
                    for h in range(HPC):
                        hsl = slice(h * DH, (h + 1) * DH)
                        nc.vector.tensor_scalar(qs[:, hsl], q_ps[:, hsl],
                                                rin[:, h:h + 1], None,
                                                ALU.mult)
                    for hh in range(HPC):
                        cs = slice(hh * DH, (hh + 1) * DH)
                        rs_ = slice((hh % 2) * DH, (hh % 2) * DH + DH)
                        tp = ps_b.tile([DH, 128], F32, tag="tp", name="tp")
                        presync_w(tp[:])
                        nc.tensor.transpose(tp[:], qs[:, cs], ident_f[:])
                        nc.vector.tensor_copy(qT[hh // 2][rs_, ts_], tp[:])
                # ---- kp pass (reuses act slots) ----
                x_sb = [load_dec3(actpool, g8_ap("X_hi", c * 128,
                                                 (c + 1) * 128),
                                  g8_ap("X_lo", c * 128, (c + 1) * 128),
                                  (128, T), SAX, f"xt{c}", tag=f"act{c}")
                        for c in range(NCH)]
                wk_sb = [load_dec3(qkpool,
                                   pc_ranges("WK_hi", c * 128,
                                             (c + 1) * 128),
                                   pc_ranges("WK_lo", c * 128,
                                             (c + 1) * 128),
                                   (128, HPC * DH), SW, f"wk{c}",
                                   tag=f"wx{c}")
                         for c in range(NCH)]
                for tb in range(NTB):
                    ts_ = slice(tb * 128, (tb + 1) * 128)
                    kp_ps = ps_b.tile([128, HPC * DH], F32, tag="proj",
                                      name="kp_ps")
                    presync_w(kp_ps[:])
                    for c in range(NCH):
                        nc.tensor.matmul(kp_ps[:], x_sb[c][:, ts_],
                                         wk_sb[c][:],
                                         start=(c == 0), stop=False)
                    nc.tensor.matmul(kp_ps[:], ones_f[:, 0:128],
                                     wkb_f[:],
                                     start=False, stop=True)
                    kp_f = work.tile([128, HPC * DH], F32, tag="kf_sb",
                                     name="kp_f")
                    nc.scalar.copy(kp_f[:], kp_ps[:])
                    nc.scalar.copy(kp_slab[tb][:], kp_ps[:])
                    for hh in range(HPC):
                        cs = slice(hh * DH, (hh + 1) * DH)
                        rs_ = slice((hh % 2) * DH, (hh % 2) * DH + DH)
                        tp2 = ps_b.tile([DH, 128], F32, tag="tp", name="tp2")
                        presync_w(tp2[:])
                        nc.tensor.transpose(tp2[:], kp_f[:, cs], ident_f[:])
                        nc.vector.tensor_copy(kT[hh // 2][rs_, ts_], tp2[:])
            actpool_cm.__exit__(None, None, None)

            # ---------------- stage A': build wedge mats on device ------
            mqd_sb = [wpool.tile([128, 128], F32, name=f"mqd{h}")
                      for h in range(HPC)]
            mkd_sb = [wpool.tile([128, 128], F32, name=f"mkd{h}")
                      for h in range(HPC)]
            with tc.tile_pool(name="ps_w", bufs=2, space="PSUM") as ps_w:
                for h in range(HPC):
                    for (gsb, bsb, dst) in ((gq_sb, biasq_sb, mqd_sb),
                                            (gk_sb, biask_sb, mkd_sb)):
                        mt = tiny.tile([DH, DH], F32, tag="mt", name="mt")
                        nc.vector.tensor_scalar(mt[:], ident_f[0:DH, 0:DH],
                                                bsb[:, h:h + 1], None,
                                                ALU.mult)
                        nc.vector.tensor_tensor(mt[:], mt[:], gsb[:],
                                                ALU.add)
                        presync_r(mt[:])
                        md_ps = ps_w.tile([DH, 128], F32, tag="md",
                                          name="md_ps")
                        presync_w(md_ps[:])
                        nc.tensor.matmul(md_ps[:], mt[:], dup_sb[:],
                                         start=True, stop=True)
                        nc.vector.tensor_copy(dst[h][0:DH, :], md_ps[:])
                        nc.vector.tensor_copy(dst[h][DH:128, :], md_ps[:])

            # ---------------- stage D: wedge + rope --------------------
            qTr = [persist.tile([128, T], F32, name=f"qTr{p}")
                   for p in range(NPAIR)]
            kTr = [persist.tile([128, T], F32, name=f"kTr{p}")
                   for p in range(NPAIR)]
            with tc.tile_pool(name="ps_d", bufs=2, space="PSUM") as ps_d:
                for h in range(HPC):
                    pair, half = h // 2, h % 2
                    rs_ = slice(half * DH, half * DH + DH)
                    for (src, lhs, dst) in ((qT, mqd_sb[h], qTr),
                                            (kT, mkd_sb[h], kTr)):
                        xd = ps_d.tile([128, T], F32, tag="xd", name="xd")
                        presync_w(xd[:])
                        for nh in range(2):
                            ns = slice(nh * 512, (nh + 1) * 512)
                            nc.tensor.matmul(xd[:, ns], lhs[rs_, :],
                                             src[pair][rs_, ns],
                                             start=True, stop=True)
                        xr = work.tile([128, T], F32, tag="xrope",
                                       name="xr")
                        nc.vector.tensor_tensor(xr[:], xd[:], frope_sb[:],
                                                ALU.mult)
                        rr = ps_d.tile([DH, T], F32, tag="rr", bufs=1,
                                       name="rr")
                        presync_w(rr[:])
                        for nh in range(2):
                            ns = slice(nh * 512, (nh + 1) * 512)
                            nc.tensor.matmul(rr[:, ns], padd_sb[:],
                                             xr[:, ns],
                                             start=True, stop=True)
                        nc.vector.tensor_copy(dst[pair][rs_, :], rr[:])
            qkpool_cm.__exit__(None, None, None)

            # ---------------- stage E: per-head attention --------------
            ctx_slab = [persist.tile([128, T], FP16, name=f"ctx{p}")
                        for p in range(NPAIR)]
            with (
                tc.tile_pool(name="ps_e1", bufs=1, space="PSUM") as ps_e1,
                tc.tile_pool(name="ps_e2", bufs=2, space="PSUM") as ps_e2,
                tc.tile_pool(name="ps_e3", bufs=1, space="PSUM") as ps_e3,
            ):
                for h in range(HPC):
                    pair, half = h // 2, h % 2
                    rs_ = slice(half * DH, half * DH + DH)
                    hsl = slice(h * DH, (h + 1) * DH)
                    kpa = [ework.tile([128, DH + 1], FP16, tag=f"kpa{j}",
                                      name=f"kpa{h}_{j}")
                           for j in range(NTB)]
                    for j in range(NTB):
                        nc.scalar.copy(kpa[j][:, 0:DH], kp_slab[j][:, hsl])
                        nc.vector.memset(kpa[j][:, DH:DH + 1], 1.0)
                    for i in range(NTB):
                        L = (i + 1) * 128
                        ts_ = slice(i * 128, (i + 1) * 128)
                        s_ps = ps_e1.tile([128, 1024], F32, tag="s_ps",
                                          name="s_ps")
                        presync_w(s_ps[:])
                        for n0 in range(0, L, 512):
                            n1 = min(n0 + 512, L)
                            nc.tensor.matmul(s_ps[:, n0:n1],
                                             qTr[pair][rs_, ts_],
                                             kTr[pair][rs_, n0:n1],
                                             start=True, stop=True)
                        nc.vector.tensor_tensor(s_ps[:, ts_], s_ps[:, ts_],
                                                tri_sb[:], ALU.add)
                        # top-12 threshold on raw scores
                        m8a = tiny.tile([128, 8], F32, tag="m8a", name="m8a")
                        m8b = tiny.tile([128, 8], F32, tag="m8b", name="m8b")
                        nc.vector.max(m8a[:], s_ps[:, 0:L])
                        r1f = ework.tile([128, 1024], F32, tag="r1f",
                                         name="r1f")
                        nc.vector.match_replace(r1f[:, 0:L], m8a[:],
                                                s_ps[:, 0:L], -1e30)
                        nc.vector.max(m8b[:], r1f[:, 0:L])
                        th_f = m8b[:, 3:4]
                        msk = ework.tile([128, 1024], FP16, tag="msk",
                                         name="msk")
                        nc.vector.tensor_scalar(msk[:, 0:L], s_ps[:, 0:L],
                                                th_f, None, ALU.is_ge)
                        e_sb = ework.tile([128, 1024], F32, tag="e_sb",
                                          name="e_sb")
                        zrow = tiny.tile([128, 1], F32, tag="zrow",
                                         name="zrow")
                        nc.scalar.activation(e_sb[:, 0:L], s_ps[:, 0:L],
                                             ACTF.Exp, accum_out=zrow[:])
                        w_sb = ework.tile([128, 1024], FP16, tag="w_sb",
                                          name="w_sb")
                        nc.gpsimd.tensor_tensor(w_sb[:, 0:L], e_sb[:, 0:L],
                                                msk[:, 0:L], ALU.mult)
                        mk_ps = ps_e3.tile([128, DH + 1], F32, tag="mk_ps",
                                           name="mk_ps")
                        for j in range(i + 1):
                            js = slice(j * 128, (j + 1) * 128)
                            wt_ps = ps_e2.tile([128, 128], FP16, tag="sm",
                                               name="wt_ps")
                            nc.tensor.transpose(wt_ps[:], w_sb[:, js],
                                                ident_h[:])
                            wt_sb = ework.tile([128, 128], FP16, tag="wt_sb",
                                               name="wt_sb")
                            nc.scalar.copy(wt_sb[:], wt_ps[:])
                            nc.tensor.matmul(mk_ps[:], wt_sb[:], kpa[j][:],
                                             start=(j == 0), stop=(j == i))
                        zf = tiny.tile([128, 1], F32, tag="zf", name="zf")
                        nc.vector.tensor_scalar(zf[:], zrow[:],
                                                esink_sb[:, h:h + 1],
                                                None, ALU.add)
                        den = tiny.tile([128, 1], F32, tag="den", name="den")
                        nc.vector.scalar_tensor_tensor(
                            den[:], zf[:], 1e-9, mk_ps[:, DH:DH + 1],
                            ALU.mult, ALU.add)
                        nu = tiny.tile([128, 1], F32, tag="nu", name="nu")
                        nc.vector.reciprocal(nu[:], den[:])
                        rz = tiny.tile([128, 1], F32, tag="rz", name="rz")
                        nc.vector.reciprocal(rz[:], zf[:])
                        mkn = tiny.tile([128, DH], FP16, tag="mkn",
                                        name="mkn")
                        nc.vector.tensor_scalar(mkn[:], mk_ps[:, 0:DH],
                                                nu[:], None, ALU.mult)
                        mt_ps = ps_e2.tile([DH, 128], FP16, tag="sm",
                                           name="mt_ps")
                        nc.tensor.transpose(mt_ps[:], mkn[:], ident_h[:])
                        mknT = tiny.tile([DH, 128], FP16, tag="mknT",
                                         name="mknT")
                        nc.scalar.copy(mknT[:], mt_ps[:])
                        h_ps = ps_e3.tile([128, 4 * DH], F32, tag="h_ps",
                                          name="h_ps")
                        nc.tensor.matmul(h_ps[:], mknT[:], fcw_h[:],
                                         start=True, stop=False)
                        nc.tensor.matmul(h_ps[:], ones_h[:, 0:128], fcb_h[:],
                                         start=False, stop=True)
                        t1 = work.tile([128, 4 * DH], FP16, tag="t1",
                                       name="t1")
                        nc.vector.tensor_scalar(t1[:], h_ps[:], 0.75, 1.0,
                                                ALU.mult, ALU.add)
                        hsq = work.tile([128, 4 * DH], FP16, tag="hsq",
                                        name="hsq")
                        nc.scalar.activation(hsq[:], h_ps[:], ACTF.Square)
                        g = work.tile([128, 4 * DH], FP16, tag="g", name="g")
                        nc.vector.tensor_tensor(g[:], hsq[:], t1[:], ALU.mult)
                        gsq = work.tile([128, 4 * DH], FP16, tag="gsq",
                                        name="gsq")
                        ssq2 = tiny.tile([128, 1], F32, tag="ssq2",
                                         name="ssq2")
                        nc.scalar.activation(gsq[:], g[:], ACTF.Square,
                                             accum_out=ssq2[:])
                        nc.vector.tensor_scalar(ssq2[:], ssq2[:],
                                                1.0 / (4 * DH), EPS,
                                                ALU.mult, ALU.add)
                        nc.scalar.activation(ssq2[:], ssq2[:], ACTF.Sqrt)
                        ni = tiny.tile([128, 1], F32, tag="ni", name="ni")
                        nc.vector.reciprocal(ni[:], ssq2[:])
                        nsc = tiny.tile([128, 1], F32, tag="nsc", name="nsc")
                        nc.vector.tensor_scalar(nsc[:], ni[:], SIG_C, None,
                                                ALU.mult)
                        sig = work.tile([128, 4 * DH], FP16, tag="sig",
                                        name="sig")
                        nc.scalar.activation(sig[:], g[:], ACTF.Sigmoid,
                                             scale=nsc[:])
                        u = work.tile([128, 4 * DH], FP16, tag="u", name="u")
                        nc.vector.scalar_tensor_tensor(u[:], g[:], ni[:],
                                                       sig[:], ALU.mult,
                                                       ALU.mult)
                        ot_ps = ps_e3.tile([DH, 128], F32, tag="ot_ps",
                                           name="ot_ps")
                        for ub in range(2):
                            us = slice(ub * 128, (ub + 1) * 128)
                            ut_ps = ps_e2.tile([128, 128], FP16, tag="sm",
                                               name="ut_ps")
                            nc.tensor.transpose(ut_ps[:], u[:, us],
                                                ident_h[:])
                            utsb = work.tile([128, 128], FP16, tag="utsb",
                                             name="utsb")
                            nc.scalar.copy(utsb[:], ut_ps[:])
                            nc.tensor.matmul(ot_ps[:], pjw_h[ub][:], utsb[:],
                                             start=(ub == 0), stop=False)
                        rzb = tiny.tile([128, 1], FP16, tag="rzb", name="rzb")
                        nc.vector.tensor_copy(rzb[:], rz[:])
                        rzt_ps = ps_e2.tile([1, 128], FP16, tag="sm",
                                            name="rzt_ps")
                        nc.tensor.transpose(rzt_ps[:], rzb[:], ident_h[:])
                        rzrow = tiny.tile([1, 128], FP16, tag="rzrow",
                                          name="rzrow")
                        nc.scalar.copy(rzrow[:], rzt_ps[:])
                        nc.tensor.matmul(ot_ps[:], pjb_h[:], ones_h[:, 0:128],
                                         start=False, stop=False)
                        nc.tensor.matmul(ot_ps[:], vsink_h[:, hsl], rzrow[:],
                                         start=False, stop=True)
                        nc.scalar.copy(ctx_slab[pair][rs_, ts_], ot_ps[:])

            # ---------------- stage F: output projection + RS ----------
            with (
                tc.tile_pool(name="ps_f", bufs=2, space="PSUM") as ps_f,
                tc.tile_pool(name="fpool", bufs=2) as fpool,
            ):
                for ob in range(NCH):
                    obs = slice(ob * 128, (ob + 1) * 128)
                    y_ps = ps_f.tile([128, T], F32, tag="y_ps", name="y_ps")
                    for p in range(NPAIR):
                        for nh in range(2):
                            ns = slice(nh * 512, (nh + 1) * 512)
                            nc.tensor.matmul(y_ps[:, ns], wop_h[p][:, obs],
                                             ctx_slab[p][:, ns],
                                             start=(p == 0), stop=False)
                    for nh in range(2):
                        ns = slice(nh * 512, (nh + 1) * 512)
                        nc.tensor.matmul(y_ps[:, ns], yb_h[:, obs],
                                         ones_h[:, ns],
                                         start=False, stop=True)
                    y_sb = fpool.tile([128, T], F32, tag="y_sb", name="y_sb")
                    nc.scalar.copy(y_sb[:], y_ps[:])
                    nc.sync.dma_start(y_bounce[obs, :], y_sb[:])
                nc.gpsimd.collective_compute(
                    "ReduceScatter", ALU.add,
                    ins=[y_bounce.ap().opt()],
                    outs=[y_rs.ap().opt()],
                    replica_groups=[list(range(N_CORES))],
                )
                rs_sb = fpool.tile([C // N_CORES, T], F32, tag="y_sb",
                                   name="rs_sb")
                nc.sync.dma_start(rs_sb[:], y_rs[:, :])
                out_sb = fpool.tile([C // N_CORES, T], FP16, tag="o16",
                                    name="out_sb")
                nc.scalar.copy(out_sb[:], rs_sb[:])
                nc.sync.dma_start(out[:, :], out_sb[:])
    nc.finalize()
    return nc


_NC_CACHE = {}


def kernel(**inputs):
    in_maps = _host_prep(**inputs)
    if "nc" not in _NC_CACHE:
        _NC_CACHE["nc"] = build_kernel()
    nc = _NC_CACHE["nc"]
    res = run_bass_kernel_spmd(nc, in_maps, core_ids=list(range(N_CORES)))
    slabs = [res.results[c]["out"] for c in range(N_CORES)]
    yT = np.concatenate(slabs, axis=0).astype(np.float32)
    return np.ascontiguousarray(yT.T)[None].astype(np.float32)
